# revision 70
# baseline (speedup 1.0000x reference)
"""Trainium2 Bass kernel for nn_MultiHeadAttention (B=2, S=2048, D=1024, H=16).

Sharding: 8 cores = 2 batches x 4 head-groups (4 heads per core, tensor
parallel over heads). Each core computes, for its batch b and its 4 heads:
  QT/KT = (x @ W.T).T projections in transposed layout [256, 2048]
  V     = value @ wv.T in normal layout, augmented with a ones column (Z trick)
  E^T   = exp(scoresT) tiles [k,q] directly from matmul (no max subtraction;
          scores are O(1) for this module so exp is safe, and masked entries
          use a multiplicative 0/1 mask so they are exactly 0)
  outT  = V_aug.T @ E^T accumulated over k tiles -> row 64 carries Z = sum(E)
  ffT   = wff_rows-partial @ (attn_outT * 1/Z)   as [1024, 2048]
Host sums the 4 partial ffT per batch, adds bff, and transposes back.

v2 performance changes vs baseline (205us -> ~181us):
  - host repacks all DRAM tensors partition-major so each DMA is a single
    128-row descriptor (DIRECT2D desc-gen 75us -> ~10us of sequencer time)
  - x loads split per (tg, kc-half) tiles in priority order: attention(0)
    unblocks after only tg0's 3MB instead of the full 12.6MB
  - a few warm-up matmuls spin the PE HAM clock-gate during initial DMA
    (more is counterproductive: the PE is power/thermal-limited, so junk
    rows steal clock budget from real matmuls)
  - AV matmuls causally trimmed like the score matmuls (et memsets dropped)
  - Z path: copy Z row to SBUF -> reciprocal (SBUF; PSUM input to the
    custom-DVE recip is silently broken on HW) -> gpsimd partition_broadcast
    (SBUF->SBUF, ~1us, replaces the DRAM bounce) -> one fused multiply
    op(PSUM) * zb -> at_g bf16 (TensorTensor needs BOTH SB inputs to share
    the base partition, hence zb is [128, .] sliced at po)
  - outT in bf16 (halves output DMA); host upcasts, sums partials, adds bff
"""

import sys

sys.path.insert(0, "/opt/trn_rl_repo")

import ml_dtypes
import numpy as np

import concourse.bass as bass
import concourse.mybir as mybir
import concourse.tile as tile
from concourse import bacc
from concourse.bass_utils import run_bass_kernel_spmd

P = 128
B, S, D, H = 2, 2048, 1024, 16
DH = D // H  # 64
NCORES = 8
GPB = NCORES // B  # cores (head groups) per batch = 4
HPC = H // GPB  # heads per core = 4
HD = HPC * DH  # projected cols per core = 256
F32 = mybir.dt.float32
F32R = mybir.dt.float32r
BF16 = mybir.dt.bfloat16
FP8 = mybir.dt.float8e4
NPFP8 = ml_dtypes.float8_e4m3fn
WSCALE = 32.0  # fp8 V-weight pre-scale (keeps wv in e4m3 normal range)
# q/k stay bf16 unscaled; the 1/sqrt(DH) score factor is folded into exp
SSCALE = 1.0 / 8.0
QGW = 512  # q-group width (psum free dim)
AF = mybir.ActivationFunctionType
NPBF16 = ml_dtypes.bfloat16
NWARM = 6  # PE clock warm-up matmuls issued during the initial DMA
INTERLEAVE_JUNK = False  # extra junk between tg0 proj groups (power cost!)
ZBCAST = "gpsimd"  # "gpsimd" (partition_broadcast) | "bounce" (DRAM roundtrip)
AV_TRIM = True  # causally trim AV matmul column ranges
# "safe": copy z + copy at, all SBUF ops (baseline semantics)
# "fusedmul": copy z + recip SBUF, but at-multiply reads op PSUM directly
# "fused": recip and multiply both read op PSUM directly
EPILOGUE = "fusedmul"

_PROG_CACHE: dict = {}


def build_program(variant: str, use_bias: bool, s=S, d=D, hpc=HPC,
                  n_devices=NCORES):
    """variant: 'causal' | 'dense' | 'generic'. Returns compiled Bacc."""
    assert variant in ("causal", "dense", "generic")
    kc_n = d // P           # contraction chunks over model dim
    tt = s // P             # token tiles
    hd = hpc * DH           # per-core projected width
    dc_n = hd // P          # dout chunks for QT/KT (and hd chunks for ff)
    tg_n = s // QGW         # token/q groups
    tpg = QGW // P          # token tiles per group (4)
    zw = hpc * QGW          # z columns per qg-pair tile
    npair = tg_n // 2       # tg pairs (x residency granularity)
    hkc = kc_n // 2         # kc per half

    nc = bacc.Bacc("TRN2", target_bir_lowering=False, debug=False,
                   num_devices=n_devices)

    def din(name, shape, dt=BF16):
        return nc.dram_tensor(name, list(shape), dt, kind="ExternalInput").ap()

    # all host tensors pre-packed partition-major: [P, ...contiguous...].
    # Everything bf16: fp8 x/w was tried but the first 512 q tokens average
    # over too few k values to wash out the ~6% e4m3 quantization noise
    # (absmax err 0.024/core vs the 0.033 gate). The 1/sqrt(DH) score
    # factor is applied via the exp activation's scale operand.
    xqH = din("xqH", (P, kc_n * s))
    xkH = din("xkH", (P, kc_n * s))
    xvH = din("xvH", (P, kc_n * s))
    wqH = din("wqH", (P, kc_n * hd))
    wkH = din("wkH", (P, kc_n * hd))
    wvH = din("wvH", (P, kc_n * hd))
    wffH = din("wffH", (P, dc_n * d))
    if use_bias:
        bq = din("bq", (hd,), F32)   # pre-scaled by 1/sqrt(DH) on host
        bk = din("bk", (hd,), F32)
        bv = din("bv", (1, hd))
        onesb = din("onesb", (1, P))
    if variant == "causal":
        dmask = din("dmask", (P, P))  # [k, q]: 1 if k <= q else 0
    if variant == "generic":
        mbT = din("mbT", (s, s), F32)  # mask[b,0].T * -1e9, [k, q] layout
    # output packed [P, nck, qg, QGW] contiguous per partition (bf16: host
    # upcasts and sums the 4 partials in f32)
    outT = nc.dram_tensor("outT", [P, kc_n * tg_n * QGW], BF16,
                          kind="ExternalOutput").ap()
    zdr = (nc.dram_tensor("zdr", [tg_n, zw], F32).ap()
           if ZBCAST == "bounce" else None)

    with tile.TileContext(nc) as tc:
        with (
            nc.allow_low_precision(reason="bf16 matmul chain; psum stays fp32"),
            tc.tile_pool(name="consts", bufs=1) as consts,
            tc.tile_pool(name="xin", bufs=1) as xin,
            tc.tile_pool(name="acts", bufs=1) as acts,
            tc.tile_pool(name="epool", bufs=8) as epool,
            tc.tile_pool(name="opool", bufs=4) as opool,
            tc.tile_pool(name="ps", bufs=1, space="PSUM") as ps,
        ):
            # PE clock warm-up: rank-1 junk matmuls fed by memset tiles (no
            # DMA dependency, so they start ~immediately). Results are never
            # read; they keep the PE HAM busy-window full so real matmuls
            # start at 2.4GHz instead of 1.2GHz.
            wm_l = wm_r = None
            _wn = [0]

            def junk(n):
                """Emit n PE warm-up matmuls (dep-free except memset tiles).

                They keep the in-order PE queue busy across DMA-wait stalls
                during the HBM-bound load phase and hold the HAM clock-gate
                at 2.4GHz."""
                if wm_l is None:
                    return
                warm_ps = ps.tile([P, 2 * QGW], F32, tag="mmw", bufs=2,
                                  name=f"warm{_wn[0]}")
                _wn[0] += 1
                for _ in range(n):
                    nc.tensor.matmul(warm_ps[:, :QGW], lhsT=wm_l[:],
                                     rhs=wm_r[:], start=True, stop=True)

            if NWARM:
                wm_l = consts.tile([1, P], BF16, tag="wm_l")
                wm_r = consts.tile([1, QGW], BF16, tag="wm_r")
                nc.gpsimd.memset(wm_l[:], 1.0)
                nc.gpsimd.memset(wm_r[:], 1.0)
                junk(NWARM)

            # ---- constant / weight loads (each one 128-row contiguous) ----
            wq_sb = consts.tile([P, kc_n, hd], BF16, tag="wq")
            wk_sb = consts.tile([P, kc_n, hd], BF16, tag="wk")
            wv_sb = consts.tile([P, kc_n, hd], BF16, tag="wv")
            wff_sb = consts.tile([P, dc_n, d], BF16, tag="wff")
            nc.sync.dma_start(wq_sb[:], wqH.rearrange("p (c m) -> p c m", m=hd))

            # x residency: tiles split per (tg, kc-half) so projections
            # gate on exactly the pieces they need; attention(0) only
            # requires tg0's 3MB.
            def xtiles(nm, dt=BF16):
                return [[acts.tile([P, hkc, QGW], dt,
                                   tag=f"{nm}{tg}{hf}",
                                   name=f"{nm}_{tg}_{hf}")
                         for hf in range(2)] for tg in range(tg_n)]

            xq_sb = xtiles("xq")
            xk_sb = xtiles("xk")
            xv_sb = xtiles("xv")

            def load_x_piece(eng, sb, xdram, tg, hf):
                src = xdram.rearrange("p (c s) -> p c s", s=s)
                eng.dma_start(
                    sb[tg][hf][:],
                    src[:, hf * hkc:(hf + 1) * hkc,
                        tg * QGW:(tg + 1) * QGW])

            # priority order: everything tg0 first (q, k, then v) so
            # proj(tg0) and attention(0) unblock earliest; later tgs after.
            load_x_piece(nc.sync, xq_sb, xqH, 0, 0)
            load_x_piece(nc.scalar, xk_sb, xkH, 0, 0)
            load_x_piece(nc.sync, xq_sb, xqH, 0, 1)
            load_x_piece(nc.scalar, xk_sb, xkH, 0, 1)
            nc.sync.dma_start(wk_sb[:], wkH.rearrange("p (c m) -> p c m",
                                                      m=hd))
            nc.sync.dma_start(wv_sb[:], wvH.rearrange("p (c m) -> p c m",
                                                      m=hd))
            load_x_piece(nc.scalar, xv_sb, xvH, 0, 0)
            load_x_piece(nc.scalar, xv_sb, xvH, 0, 1)
            for tg in range(1, tg_n):
                load_x_piece(nc.sync, xq_sb, xqH, tg, 0)
                load_x_piece(nc.scalar, xk_sb, xkH, tg, 0)
                load_x_piece(nc.sync, xq_sb, xqH, tg, 1)
                load_x_piece(nc.scalar, xk_sb, xkH, tg, 1)
                load_x_piece(nc.scalar, xv_sb, xvH, tg, 0)
                load_x_piece(nc.scalar, xv_sb, xvH, tg, 1)
                if tg == 2:
                    nc.sync.dma_start(
                        wff_sb[:],
                        wffH.rearrange("p (c m) -> p c m", m=d))

            if use_bias:
                bq_sb = consts.tile([P, dc_n], F32, tag="bq")
                bk_sb = consts.tile([P, dc_n], F32, tag="bk")
                nc.sync.dma_start(bq_sb[:], bq.rearrange("(c p) -> p c", p=P))
                nc.sync.dma_start(bk_sb[:], bk.rearrange("(c p) -> p c", p=P))
                bv_sb = consts.tile([1, hd], BF16, tag="bv")
                nc.sync.dma_start(bv_sb[:], bv[:])
                onesb_sb = consts.tile([1, P], BF16, tag="onesb")
                nc.sync.dma_start(onesb_sb[:], onesb[:])
            if variant == "causal":
                dmask_sb = consts.tile([P, P], BF16, tag="dmask")
                nc.sync.dma_start(dmask_sb[:], dmask[:])

            # per-group activation tiles (split so the scheduler can
            # pipeline groups without whole-tile false dependencies)
            qT_g = [acts.tile([P, dc_n, QGW], BF16, tag=f"qT{g}",
                              name=f"qT_{g}") for g in range(tg_n)]
            kT_g = [acts.tile([P, dc_n, QGW], BF16, tag=f"kT{g}",
                              name=f"kT_{g}") for g in range(tg_n)]
            va_g = [acts.tile([P, tpg, hpc * (DH + 1)], BF16, tag=f"va{g}",
                              name=f"va_{g}") for g in range(tg_n)]
            at_g = [acts.tile([P, dc_n, QGW], BF16, tag=f"at{g}",
                              name=f"at_{g}") for g in range(tg_n)]


            # ones-column memsets for the V/Z trick: no deps, issue up front
            for tg in range(tg_n):
                nc.gpsimd.memset(
                    va_g[tg].rearrange("p t (h e) -> p t h e",
                                       e=DH + 1)[:, :, :, DH], 1.0)

            def proj_qk_units(tg, w_sb, b_sb, x_sb, dest, out):
                """Append filler units: 2 half-chains per dc."""
                cell = {}
                for dc in range(dc_n):
                    for half in range(2):
                        def chain(tg=tg, dc=dc, half=half, w_sb=w_sb,
                                  x_sb=x_sb, b_sb=b_sb, dest=dest):
                            if half == 0:
                                cell[dc] = ps.tile([P, QGW], F32, tag="pacc",
                                                   bufs=2,
                                                   name=f"pp_{tg}_{dc}")
                            pp = cell[dc]
                            k0 = half * hkc
                            for kc in range(k0, k0 + hkc):
                                nc.tensor.matmul(
                                    pp[:],
                                    lhsT=w_sb[:, kc, dc * P:(dc + 1) * P],
                                    rhs=x_sb[tg][half][:, kc - k0, :],
                                    start=(kc == 0),
                                    stop=(kc == kc_n - 1),
                                )
                            if half == 1:
                                if use_bias:
                                    nc.scalar.activation(
                                        dest[:, dc, :], pp[:], AF.Identity,
                                        bias=b_sb[:, dc:dc + 1])
                                else:
                                    nc.vector.tensor_copy(dest[:, dc, :],
                                                          pp[:])
                        out.append(chain)

            def proj_v_units(tg, out):
                for ti in range(tpg):
                    def v_unit(tg=tg, ti=ti):
                        t = tg * tpg + ti
                        vp = ps.tile([P, QGW], F32, tag="pacc", bufs=2,
                                     name=f"vp_{t}")
                        if use_bias:
                            nc.tensor.matmul(vp[:, :hd],
                                             lhsT=onesb_sb[0:1, :],
                                             rhs=bv_sb[:, :], start=True,
                                             stop=False)
                        for kc in range(kc_n):
                            hf, kcl = kc // hkc, kc % hkc
                            nc.tensor.matmul(
                                vp[:, :hd],
                                lhsT=xv_sb[tg][hf][:, kcl,
                                                   ti * P:(ti + 1) * P],
                                rhs=wv_sb[:, kc, :],
                                start=(kc == 0 and not use_bias),
                                stop=(kc == kc_n - 1),
                            )
                        nc.vector.tensor_copy(
                            va_g[tg][:, ti].rearrange(
                                "p (h e) -> p h e", e=DH + 1)[:, :, :DH],
                            vp[:, :hd].rearrange("p (h e) -> p h e", e=DH))
                    out.append(v_unit)

            def ff_units(qg, out):
                for nck in range(kc_n):
                    def ff_unit(qg=qg, nck=nck):
                        fp = ps.tile([P, QGW], F32, tag="pacc", bufs=2,
                                     name=f"fp_{nck}_{qg}")
                        for dc in range(dc_n):
                            nc.tensor.matmul(
                                fp[:],
                                lhsT=wff_sb[:, dc, nck * P:(nck + 1) * P],
                                rhs=at_g[qg][:, dc, :],
                                start=(dc == 0),
                                stop=(dc == dc_n - 1),
                            )
                        ot = opool.tile([P, QGW], BF16, tag="otile",
                                        name=f"ot_{nck}_{qg}")
                        # last group's copies ride the then-idle scalar
                        # engine so the vector queue can't delay the tail
                        if qg == tg_n - 1:
                            nc.scalar.copy(ot[:], fp[:])
                        else:
                            nc.vector.tensor_copy(ot[:], fp[:])
                        nc.sync.dma_start(
                            outT[:, (nck * tg_n + qg) * QGW:
                                 (nck * tg_n + qg + 1) * QGW], ot[:])
                    out.append(ff_unit)

            def run_units(units, n=None):
                k = len(units) if n is None else min(n, len(units))
                for _ in range(k):
                    units.popleft()()

            def attention(qg, fillers):
                kmax = (qg + 1) * tpg if variant == "causal" else tt
                PW = 2  # score tiles batched per exp
                nquad = kmax // PW
                for h in range(hpc):
                    po = (h * DH) % P
                    dch = (h * DH) // P
                    op = ps.tile([P, QGW], F32, tag="opacc", bufs=2,
                                 name=f"op_{h}_{qg}")
                    ets = [None] * nquad

                    def emit_scores(qd):
                        sp = ps.tile([P, PW * QGW], F32, tag="mmw", bufs=2,
                                     name=f"sp_{h}_{qg}_{qd}")
                        for j in range(PW):
                            kt = qd * PW + j
                            off = (max(0, kt * P - qg * QGW)
                                   if variant == "causal" else 0)
                            kg, kx = kt // tpg, kt % tpg
                            kh = kT_g[kg][po:po + DH, dch,
                                          kx * P:(kx + 1) * P]
                            nc.tensor.matmul(
                                sp[:, j * QGW + off:(j + 1) * QGW],
                                lhsT=kh,
                                rhs=qT_g[qg][po:po + DH, dch, off:],
                                start=True,
                                stop=True,
                            )
                            if variant == "generic":
                                mb_sb = xin.tile([P, QGW], F32, tag="mstream",
                                                 bufs=4,
                                                 name=f"mb_{h}_{qg}_{kt}")
                                nc.sync.dma_start(
                                    mb_sb[:],
                                    mbT[kt * P:(kt + 1) * P,
                                        qg * QGW:(qg + 1) * QGW])
                                nc.vector.tensor_add(
                                    sp[:, j * QGW:(j + 1) * QGW],
                                    sp[:, j * QGW:(j + 1) * QGW], mb_sb[:])
                        et = epool.tile([P, PW * QGW], BF16, tag="etile",
                                        name=f"et_{h}_{qg}_{qd}")
                        offs = [(max(0, (qd * PW + j) * P - qg * QGW)
                                 if variant == "causal" else 0)
                                for j in range(PW)]
                        if not any(offs):
                            nc.scalar.activation(et[:], sp[:], AF.Exp,
                                                 scale=SSCALE)
                        else:
                            for j in range(PW):
                                o = j * QGW + offs[j]
                                nc.scalar.activation(
                                    et[:, o:(j + 1) * QGW],
                                    sp[:, o:(j + 1) * QGW], AF.Exp,
                                    scale=SSCALE)
                        if variant == "causal":
                            for j in range(PW):
                                kt = qd * PW + j
                                off = kt * P - qg * QGW
                                if off < 0:
                                    continue
                                if off and not AV_TRIM:
                                    nc.gpsimd.memset(
                                        et[:, j * QGW:j * QGW + off], 0.0)
                                nc.vector.tensor_mul(
                                    et[:, j * QGW + off:j * QGW + off + P],
                                    et[:, j * QGW + off:j * QGW + off + P],
                                    dmask_sb[:])
                        ets[qd] = et

                    def emit_av(qd):
                        et = ets[qd]
                        for j in range(PW):
                            kt = qd * PW + j
                            off = (max(0, kt * P - qg * QGW)
                                   if variant == "causal" and AV_TRIM else 0)
                            kg, kx = kt // tpg, kt % tpg
                            nc.tensor.matmul(
                                op[:DH + 1, off:],
                                lhsT=va_g[kg][:, kx, h * (DH + 1):
                                              (h + 1) * (DH + 1)],
                                rhs=et[:, j * QGW + off:(j + 1) * QGW],
                                start=(kt == 0),
                                stop=(kt == kmax - 1),
                            )
                        ets[qd] = None

                    emit_scores(0)
                    for qd in range(1, nquad):
                        emit_scores(qd)
                        run_units(fillers, 1)
                        emit_av(qd - 1)
                    emit_av(nquad - 1)
                    run_units(fillers, 1)
                    # Z path: 1/Z, broadcast to DH partitions, multiply into
                    # at (bf16)
                    hs = slice(h * QGW, (h + 1) * QGW)
                    zi_t = acts.tile([1, QGW], F32, tag="zi", bufs=4,
                                     name=f"zi_{qg}_{h}")
                    zb_t = acts.tile([P, QGW], F32, tag="zbb", bufs=4,
                                     name=f"zb_{qg}_{h}")
                    if EPILOGUE == "fused":
                        nc.vector.reciprocal_approx_fast(zi_t[0:1, :],
                                                         op[DH:DH + 1, :])
                    else:
                        z_t = acts.tile([1, QGW], F32, tag="zz", bufs=4,
                                        name=f"z_{qg}_{h}")
                        nc.vector.tensor_copy(z_t[0:1, :], op[DH:DH + 1, :])
                        nc.vector.reciprocal_approx_fast(zi_t[0:1, :],
                                                         z_t[0:1, :])
                    if ZBCAST == "gpsimd":
                        # fusedmul's PSUM in0 is exempt from the TensorTensor
                        # same-base-partition rule, so 64 partitions suffice
                        zch = DH if EPILOGUE in ("fused", "fusedmul") else P
                        nc.gpsimd.partition_broadcast(zb_t[:zch, :],
                                                      zi_t[0:1, :],
                                                      channels=zch)
                    else:
                        nc.sync.dma_start(zdr[qg:qg + 1, hs], zi_t[0:1, :])
                        nc.sync.dma_start(
                            zb_t[:],
                            zdr[qg:qg + 1, hs].to_broadcast([P, QGW]))
                    if EPILOGUE in ("fused", "fusedmul"):
                        nc.vector.tensor_mul(
                            at_g[qg][po:po + DH, dch, :], op[:DH, :],
                            zb_t[:DH, :])
                    else:
                        nc.vector.tensor_copy(
                            at_g[qg][po:po + DH, dch, :], op[:DH, :])
                        nc.vector.tensor_mul(
                            at_g[qg][po:po + DH, dch, :],
                            at_g[qg][po:po + DH, dch, :],
                            zb_t[po:po + DH, :])

            # ---- schedule over token groups ----
            # causal: attention(qg) only needs k/v groups <= qg, so proj of
            # group tg+1 and ff of group tg-1 are injected as filler units
            # between attention matmul pairs to keep PE busy during exp.
            from collections import deque
            fillers = deque()

            def queue_proj(tg):
                proj_qk_units(tg, wq_sb, bq_sb if use_bias else None,
                              xq_sb, qT_g[tg], fillers)
                proj_qk_units(tg, wk_sb, bk_sb if use_bias else None,
                              xk_sb, kT_g[tg], fillers)
                proj_v_units(tg, fillers)

            if variant == "causal":
                # tg0 proj with junk matmuls between q/k/v groups: the
                # in-order PE queue otherwise stalls while each x piece's
                # DMA lands (the load phase is HBM-bandwidth-bound)
                proj_qk_units(0, wq_sb, bq_sb if use_bias else None,
                              xq_sb, qT_g[0], fillers)
                run_units(fillers)
                if INTERLEAVE_JUNK:
                    junk(8)
                proj_qk_units(0, wk_sb, bk_sb if use_bias else None,
                              xk_sb, kT_g[0], fillers)
                run_units(fillers)
                if INTERLEAVE_JUNK:
                    junk(8)
                proj_v_units(0, fillers)
                run_units(fillers)
                if INTERLEAVE_JUNK:
                    junk(6)
                for tg in range(tg_n):
                    if tg + 1 < tg_n:
                        queue_proj(tg + 1)
                    if tg > 0:
                        ff_units(tg - 1, fillers)
                    attention(tg, fillers)
                    run_units(fillers)
                ff_units(tg_n - 1, fillers)
                run_units(fillers)
            else:
                for tg in range(tg_n):
                    queue_proj(tg)
                    run_units(fillers)
                for qg in range(tg_n):
                    if qg > 0:
                        ff_units(qg - 1, fillers)
                    attention(qg, fillers)
                    run_units(fillers)
                ff_units(tg_n - 1, fillers)
                run_units(fillers)

    nc.compile()
    return nc


def _classify_mask(mask: np.ndarray) -> str:
    m = np.asarray(mask)[:, 0]  # [B, S, S]
    if not m.any():
        return "dense"
    s = m.shape[-1]
    causal = np.triu(np.ones((s, s), dtype=m.dtype), k=1)
    if all(np.array_equal(m[b], causal) for b in range(m.shape[0])):
        return "causal"
    return "generic"


def _bf(x):
    return np.ascontiguousarray(np.ascontiguousarray(x).astype(NPBF16))


def _f8(x):
    return np.ascontiguousarray(np.ascontiguousarray(x).astype(NPFP8))


def _pack_pmajor(xT: np.ndarray) -> np.ndarray:
    """[R, C] with R = kc*P  ->  [P, kc*C] partition-major contiguous."""
    r, c = xT.shape
    kc = r // P
    return np.ascontiguousarray(
        xT.reshape(kc, P, c).transpose(1, 0, 2).reshape(P, kc * c))


def _make_in_maps(variant, query, key, value, mask, wq, bq, wk, bk, wv, bv,
                  wff, bff, use_bias):
    ws = np.float32(WSCALE)
    wqTs = _bf(wq.T)
    wkT = _bf(wk.T)
    wvT = _bf(wv.T)
    wffT = _bf(wff.T)

    xqH = [_pack_pmajor(_bf(query[b].T)) for b in range(B)]
    xkH = [_pack_pmajor(_bf(key[b].T)) for b in range(B)]
    xvH = [_pack_pmajor(_bf(value[b].T)) for b in range(B)]
    mbT = None
    if variant == "generic":
        # mask is added to the un-rescaled scores; undo the exp scale
        mbT = [np.ascontiguousarray(mask[b, 0].T * np.float32(-1e9 / SSCALE))
               for b in range(B)]

    dmask = np.tril(np.ones((P, P), np.float32)).T

    in_maps = []
    for c in range(NCORES):
        b, hg = c // GPB, c % GPB
        sl = slice(hg * HD, (hg + 1) * HD)
        m = {
            "xqH": xqH[b], "xkH": xkH[b], "xvH": xvH[b],
            "wqH": _pack_pmajor(np.ascontiguousarray(wqTs[:, sl])),
            "wkH": _pack_pmajor(np.ascontiguousarray(wkT[:, sl])),
            "wvH": _pack_pmajor(np.ascontiguousarray(wvT[:, sl])),
            "wffH": _pack_pmajor(np.ascontiguousarray(wffT[sl, :])),
        }
        if use_bias:
            m["bq"] = np.ascontiguousarray(bq[sl]).astype(np.float32)
            m["bk"] = np.ascontiguousarray(bk[sl]).astype(np.float32)
            m["bv"] = _bf(bv[sl])[None, :]
            m["onesb"] = np.ones((1, P), NPBF16)
        if variant == "causal":
            m["dmask"] = _bf(dmask)
        if variant == "generic":
            m["mbT"] = mbT[b]
        in_maps.append(m)
    return in_maps


def _unpack_out(packed: np.ndarray) -> np.ndarray:
    """[P, kc*tg*QGW] -> [D, S] (transposed output layout)."""
    kc = D // P
    tg = S // QGW
    return packed.reshape(P, kc, tg, QGW).transpose(1, 0, 2, 3).reshape(D, S)


def kernel(**inputs) -> np.ndarray:
    query = np.ascontiguousarray(inputs["query"], dtype=np.float32)
    key = np.ascontiguousarray(inputs["key"], dtype=np.float32)
    value = np.ascontiguousarray(inputs["value"], dtype=np.float32)
    mask = np.asarray(inputs["mask"], dtype=np.float32)
    wq = np.asarray(inputs["wq"], np.float32)
    bq = np.asarray(inputs["bq"], np.float32)
    wk = np.asarray(inputs["wk"], np.float32)
    bk = np.asarray(inputs["bk"], np.float32)
    wv = np.asarray(inputs["wv"], np.float32)
    bv = np.asarray(inputs["bv"], np.float32)
    wff = np.asarray(inputs["wff"], np.float32)
    bff = np.asarray(inputs["bff"], np.float32)

    variant = _classify_mask(mask)
    use_bias = bool(bq.any() or bk.any() or bv.any() or bff.any())
    pkey = (variant, use_bias)
    if pkey not in _PROG_CACHE:
        _PROG_CACHE[pkey] = build_program(variant, use_bias)
    nc = _PROG_CACHE[pkey]

    in_maps = _make_in_maps(variant, query, key, value, mask, wq, bq, wk, bk,
                            wv, bv, wff, bff, use_bias)
    res = run_bass_kernel_spmd(nc, in_maps, core_ids=list(range(NCORES)))
    out = np.empty((B, S, D), np.float32)
    for b in range(B):
        acc = _unpack_out(res.results[b * GPB]["outT"].astype(np.float32))
        for g in range(1, GPB):
            acc = acc + _unpack_out(
                res.results[b * GPB + g]["outT"].astype(np.float32))
        out[b] = acc.T + bff[None, :]
    return out


if __name__ == "__main__":
    import reference

    inputs = {k: np.asarray(v) for k, v in reference.setup_inputs().items()}
    out = kernel(**inputs)
    print("kernel out:", out.shape, out.dtype)
